# revision 24
# baseline (speedup 1.0000x reference)
"""LocallyConnected2D (B=16, 32x32, CIN=COUT=64, 3x3, pad=1) on 8 TRN2 NeuronCores.

Shard the 32 output rows across 8 cores (4 rows each); all tensors fp16 on
device (fp32 PSUM accumulate), fp32 finish on host.

Column-stationary formulation: input column c feeds the dj=2,1,0 taps of
output pixels c-1, c, c+1, so ONE matmul with stationary lhsT = x[:, c]
streams the weights of up to 3 adjacent pixels (N<=192). Row taps are
packed on the contraction axis: di=(0,1) as K=128 pairs (x rows r, r+1
stacked on partition halves = panel V(r)), di=2 as K=64 solo matmuls whose
weights ride partition half 64*(c%2) so the weight DMA stays 128-partition.
Taps that multiply the zero column padding are skipped entirely (their
weights are never sent). ~38 matmuls per (row, structure) instead of 160
tiny ones; 4 psum column strips of 8 consecutive pixels run concurrently
via tile_position.

out[b,i,j,o] = sum_{c,k} x_pad[b, i+di, j+dj, cin] * W[o,cin,i,j,3di+dj].

Host layouts (per core C, local row r, i = 4C+r, strip s = j//8, f = j%8):
  w_pairs [4, 128, 6016]: [64*di+cin, flat] = W[o,cin,i,p,dj+3di], di=0,1
  w_solo  [4, 128, 3008]: [64*(c%2)+cin, flat] = W[o,cin,i,p,6+dj]
  (flat, in matmul schedule order: c=0..31, strip pieces, pixels asc, o asc)
  xt      [384, 512]:     [rin*64+cin, j*16+b] = x_pad[b, 4C+rin, j, cin]
  out     [4, 16, 2048] fp16: [s, b, r*512 + f*64 + o] = out[b, i, 8s+f, o]

x panels V(k) [128, 512], k=0..4: partitions = x rows (k, k+1), col c at
c*16 (no padding columns needed); S5 [64, 512] = row 5. Pair lhsT =
V(r)[:, c*16:+16]; solo lhsT = row r+2: V(r+2)[0:64] / S5 (c even),
V(r+1)[64:128] (c odd).

PSUM: one [128, 512] bank per r, DVE-zeroed first (all matmuls
start=False accumulate); 4 DVE casts f32->fp16 per r into stage; streamed
[16, 1KB] output DMAs. Pair weights ride the SP HWDGE ring; solo weights,
x and out ride the ACT ring.
"""

import numpy as np

B, IH, IW, CIN = 16, 32, 32, 64
COUT, OH, OW = 64, 32, 32
NCORES, RPC = 8, 4

_NC = None


def _schedule():
    """Matmul schedule: list of q-groups, each a list of (c, s, p_lo, npix)
    pieces. Columns are interleaved across the 4 strips (c = 8s+q for
    q=0..7) so consecutive matmuls hit different PE column groups and
    overlapping accumulate regions are several instructions apart. Shared
    by the kernel builder and the host weight packer so the flat weight
    layout matches consumption order exactly."""
    groups = []
    for q in range(8):
        grp = []
        for st in range(4):
            c = 8 * st + q
            pixels = [p for p in (c - 1, c, c + 1) if 0 <= p < 32]
            run = []
            for p in pixels:
                if run and (p // 8 != run[0] // 8):
                    grp.append((c, run[0] // 8, run[0], len(run)))
                    run = []
                run.append(p)
            if run:
                grp.append((c, run[0] // 8, run[0], len(run)))
        groups.append(grp)
    return groups


def _build_nc(n_reps=1):
    import concourse.bacc as bacc
    import concourse.mybir as mybir
    import concourse.tile as tile

    f16 = mybir.dt.float16
    f32 = mybir.dt.float32
    groups = _schedule()
    ntap = sum(npix for g in groups for _, _, _, npix in g)  # 94
    nc = bacc.Bacc("TRN2", target_bir_lowering=False, debug=False)
    wp = nc.dram_tensor("w_pairs", [RPC, 128, ntap * 64], f16, kind="ExternalInput")
    wso = nc.dram_tensor("w_solo", [RPC, 128, ntap * 32], f16, kind="ExternalInput")
    xt = nc.dram_tensor("xt", [384, 512], f16, kind="ExternalInput")
    out = nc.dram_tensor("out", [4, 16, RPC * 512], f16, kind="ExternalOutput")
    wp_ap, wso_ap, xt_ap, out_ap = wp.ap(), wso.ap(), xt.ap(), out.ap()

    with tile.TileContext(nc) as tc:
        with (
            tc.tile_pool(name="wp", bufs=3) as wp_pool,
            tc.tile_pool(name="wso", bufs=3) as wso_pool,
            tc.tile_pool(name="vx", bufs=2) as vx_pool,
            tc.tile_pool(name="stage", bufs=2) as stage_pool,
            tc.tile_pool(name="psum", bufs=8, space="PSUM") as psum_pool,
        ):
            for rep in range(n_reps):
                # HBM loads only for even panels; odd panels + S5 are built
                # with on-chip SBUF->SBUF copies (no HBM bandwidth).
                vs = []
                for k in range(5):
                    v = vx_pool.tile([128, 512], f16, tag=f"v{k}")
                    vs.append(v)
                s5 = vx_pool.tile([64, 512], f16, tag="s5")
                for k in (0, 2, 4):
                    nc.scalar.dma_start(vs[k][:], xt_ap[64 * k : 64 * k + 128])
                nc.scalar.dma_start(vs[1][0:64, :], vs[0][64:128, :])
                nc.scalar.dma_start(vs[1][64:128, :], vs[2][0:64, :])
                nc.sync.dma_start(vs[3][0:64, :], vs[2][64:128, :])
                nc.sync.dma_start(vs[3][64:128, :], vs[4][0:64, :])
                nc.sync.dma_start(s5[:], vs[4][64:128, :])

                stage = stage_pool.tile([128, 2048], f16, tag="stage")
                for r in range(RPC):
                    wp_t = wp_pool.tile([128, ntap * 64], f16, tag="wp")
                    wso_t = wso_pool.tile([128, ntap * 32], f16, tag="wso")
                    h = ntap * 32
                    nc.sync.dma_start(wp_t[:, 0:h], wp_ap[r][:, 0:h])
                    nc.scalar.dma_start(wso_t[:, 0 : h // 2], wso_ap[r][:, 0 : h // 2])
                    nc.sync.dma_start(wp_t[:, h : 2 * h], wp_ap[r][:, h : 2 * h])
                    nc.scalar.dma_start(wso_t[:, h // 2 : h], wso_ap[r][:, h // 2 : h])

                    ps = psum_pool.tile([128, 512], f32, tag="ps")
                    nc.vector.memset(ps[:], 0.0)
                    off = 0
                    soff = [0, 0]
                    nmm = sum(1 for g in groups for _ in g)
                    mm_i = 0
                    for grp in groups:
                        poffs, soffs = [], []
                        for c, s, p_lo, npix in grp:
                            n = npix * 64
                            poffs.append(off)
                            soffs.append(soff[c % 2])
                            off += n
                            soff[c % 2] += n
                        # pair pass: di=(0,1), K=128
                        for (c, s, p_lo, npix), po in zip(grp, poffs):
                            n = npix * 64
                            pslice = ps[
                                32 * s : 32 * s + 16,
                                (p_lo % 8) * 64 : (p_lo % 8) * 64 + n,
                            ]
                            nc.tensor.matmul(
                                pslice,
                                vs[r][:, c * 16 : (c + 1) * 16],
                                wp_t[:, po : po + n],
                                start=False,
                                stop=False,
                                tile_position=(0, 32 * s),
                                skip_group_check=True,
                            )
                        # solo pass: di=2, K=64, weights on half 64*(c%2)
                        for (c, s, p_lo, npix), so in zip(grp, soffs):
                            n = npix * 64
                            mm_i += 1
                            pslice = ps[
                                32 * s : 32 * s + 16,
                                (p_lo % 8) * 64 : (p_lo % 8) * 64 + n,
                            ]
                            if c % 2 == 0:
                                xsrc = (s5 if r == 3 else vs[r + 2])[
                                    0:64, c * 16 : (c + 1) * 16
                                ]
                                wsrc = wso_t[0:64, so : so + n]
                                tp = (0, 32 * s)
                            else:
                                xsrc = vs[r + 1][64:128, c * 16 : (c + 1) * 16]
                                wsrc = wso_t[64:128, so : so + n]
                                tp = (64, 32 * s)
                            nc.tensor.matmul(
                                pslice,
                                xsrc,
                                wsrc,
                                start=False,
                                stop=(mm_i == nmm),
                                tile_position=tp,
                                skip_group_check=True,
                            )
                    for s in range(4):
                        nc.vector.tensor_copy(
                            stage[32 * s : 32 * s + 16, r * 512 : (r + 1) * 512],
                            ps[32 * s : 32 * s + 16, :],
                        )
                        if r % 2 == 1:
                            nc.scalar.dma_start(
                                out_ap[s][:, (r - 1) * 512 : (r + 1) * 512],
                                stage[
                                    32 * s : 32 * s + 16,
                                    (r - 1) * 512 : (r + 1) * 512,
                                ],
                            )
    nc.compile()
    return nc


def _repack_inputs(x, weight):
    x = np.asarray(x, dtype=np.float32)
    weight = np.asarray(weight, dtype=np.float32)
    sched = [piece for grp in _schedule() for piece in grp]
    ntap = sum(npix for _, _, _, npix in sched)

    # wt[i, cin, o, j, k]
    wt = np.ascontiguousarray(weight.transpose(2, 1, 0, 3, 4)).astype(np.float16)
    wpair = np.zeros((OH, 128, ntap * 64), dtype=np.float16)
    wsolo = np.zeros((OH, 128, ntap * 32), dtype=np.float16)
    off = 0
    soff = [0, 0]
    for c, s, p_lo, npix in sched:
        for e, p in enumerate(range(p_lo, p_lo + npix)):
            dj = c - p + 1
            pb = slice(off + 64 * e, off + 64 * (e + 1))
            sb = slice(soff[c % 2] + 64 * e, soff[c % 2] + 64 * (e + 1))
            wpair[:, 0:64, pb] = wt[:, :, :, p, dj]  # di=0
            wpair[:, 64:128, pb] = wt[:, :, :, p, 3 + dj]  # di=1
            half = 64 * (c % 2)
            wsolo[:, half : half + 64, sb] = wt[:, :, :, p, 6 + dj]  # di=2
        off += 64 * npix
        soff[c % 2] += 64 * npix

    xpad = np.zeros((IH + 2, CIN, IW, B), dtype=np.float16)
    xpad[1:33] = x.transpose(1, 3, 2, 0)  # [ih, c, j, b]

    in_maps = []
    for c in range(NCORES):
        in_maps.append(
            {
                "w_pairs": np.ascontiguousarray(wpair[c * RPC : (c + 1) * RPC]),
                "w_solo": np.ascontiguousarray(wsolo[c * RPC : (c + 1) * RPC]),
                "xt": np.ascontiguousarray(
                    xpad[c * RPC : c * RPC + RPC + 2].reshape(384, 512)
                ),
            }
        )
    return in_maps


def _get_nc():
    global _NC
    if _NC is None:
        _NC = _build_nc()
    return _NC


def run_spmd(in_maps, **kwargs):
    from concourse.bass_utils import run_bass_kernel_spmd

    return run_bass_kernel_spmd(
        _get_nc(), in_maps, core_ids=list(range(NCORES)), **kwargs
    )


def kernel(x, weight, bias, _results=None):
    if _results is None:
        _results = run_spmd(_repack_inputs(x, weight)).results
    arr = np.stack([r["out"] for r in _results]).astype(np.float32)
    arr = arr.reshape(NCORES, 4, 16, RPC, 8, 64)
    # arr: [core, s, b, r, f, o] -> out[b, 4*core+r, 8s+f, o]
    out = arr.transpose(2, 0, 3, 1, 4, 5).reshape(B, OH, OW, COUT)
    return out + np.asarray(bias, dtype=np.float32)[None]
